# revision 10
# baseline (speedup 1.0000x reference)
"""Trainium2 Bass kernel for nn_MultiHeadAttention_49701361549443.

Model (reference.py):
    B, L, DM, H, DK, DV = 4, 2048, 512, 8, 64, 64; MAXLEN = L
    qh/kh/vh = (q|k|v) @ (wq|wk|wv), per-head attention with a causal mask and
    a skewed relative-position bias, softmax -> p, out = p@vh @ wo + residual,
    LayerNorm(eps=1e-6).  Returns (out, p).

Sharding (8 cores): core = batch * 2 + head_half.  Each core handles one
batch and 4 of the 8 heads, all 2048 query rows -- fully uniform SPMD.
The output projection partial sums are reduced on the host, followed by
residual + LayerNorm (trivial numpy work).  p slabs concatenate per head.

Skew trick: the reference's pad+reshape causal skew is equivalent to
    bias[q, k] = R[q, (L-1) - q + k],   R = (q@wq/8) @ rel_emb[h]
which, for R stored row-major [L, L] in DRAM, is a plain 2D strided read with
row stride (L-1).  So R is computed on-chip, round-tripped through DRAM in
fp16, and read back "skewed" with a strided DMA.  Entries with k > q read
garbage from the next row; they are crushed by a -1e4 additive mask before
exp, giving exact 0.0 in the fp16 softmax numerator (matches softmax of
-inf in the reference).
"""

import math

import numpy as np

B, L, DM, H, DK, DV = 4, 2048, 512, 8, 64, 64
HPC = H // 2            # heads per core
NCORES = 8
EPS = 1e-6
P = 128                 # partitions
TQ = L // P             # query tiles of 128 rows
KC = 512                # key-chunk width (one PSUM bank of fp32)
MASKVAL = -10000.0

_CACHE = {}


def _nchunks(t):
    """512-wide key chunks needed by query tile t (causal)."""
    return (t * P) // KC + 1


def _mi_start(t):
    """First 512-wide R column chunk read by tile t (m >= L-128 - t*128)."""
    return max(0, (L - P - t * P)) // KC


def _build_program():
    import concourse.bacc as bacc
    import concourse.bass as bass
    import concourse.tile as tile
    from concourse import mybir
    from concourse.masks import make_identity

    f32 = mybir.dt.float32
    f16 = mybir.dt.float16

    nc = bacc.Bacc(
        "TRN2",
        target_bir_lowering=False,
        debug=False,
        enable_asserts=False,
        num_devices=NCORES,
    )

    # ---- I/O ----------------------------------------------------------------
    qT = nc.dram_tensor("qT", [DM, L], f16, kind="ExternalInput").ap()
    kT = nc.dram_tensor("kT", [DM, L], f16, kind="ExternalInput").ap()
    vT = nc.dram_tensor("vT", [DM, L], f16, kind="ExternalInput").ap()
    wq = nc.dram_tensor("wq", [DM, HPC * DK], f16, kind="ExternalInput").ap()
    wk = nc.dram_tensor("wk", [DM, HPC * DK], f16, kind="ExternalInput").ap()
    wv = nc.dram_tensor("wv", [DM, HPC * DV], f16, kind="ExternalInput").ap()
    wo = nc.dram_tensor("wo", [HPC * DV, DM], f16, kind="ExternalInput").ap()
    rel = nc.dram_tensor("rel", [HPC, DK, L], f16, kind="ExternalInput").ap()

    p_out = nc.dram_tensor("p_out", [HPC, L, L], f32, kind="ExternalOutput").ap()
    o_out = nc.dram_tensor("o_out", [L, DM], f32, kind="ExternalOutput").ap()

    with tile.TileContext(nc) as tc:
        with (
            tc.tile_pool(name="consts", bufs=1) as consts,
            tc.tile_pool(name="persist", bufs=1) as persist,
            tc.tile_pool(name="inload", bufs=8) as inload,
            tc.tile_pool(name="btiles", bufs=4) as btiles,
            tc.tile_pool(name="ptil", bufs=6) as ptil,
            tc.tile_pool(name="pT", bufs=3) as pTp,
            tc.tile_pool(name="small", bufs=8) as small,
            tc.tile_pool(name="rstage", bufs=4) as rstage,
            tc.tile_pool(name="pstage", bufs=2) as pstage,
            tc.tile_pool(name="psL", bufs=1, space="PSUM") as psL,
            tc.tile_pool(name="psS", bufs=3, space="PSUM") as psS,
            tc.tile_pool(name="psT", bufs=2, space="PSUM") as psT,
            tc.tile_pool(name="rdram", bufs=2, space="DRAM") as rdram,
        ):
            # ---- constants ------------------------------------------------
            ident = consts.tile([P, P], f16)
            make_identity(nc, ident)
            masks = []
            for d in (0, 128, 256, 384):
                m = consts.tile([P, KC], f16, tag=f"mask{d}", name=f"mask{d}")
                nc.gpsimd.memset(m, 0.0)
                # keep 0.0 where (d + p - j) >= 0  i.e. j <= d+p, else MASKVAL
                nc.gpsimd.affine_select(
                    out=m, in_=m,
                    compare_op=mybir.AluOpType.is_ge,
                    fill=MASKVAL, base=d,
                    pattern=[[-1, KC]], channel_multiplier=1,
                )
                masks.append(m)

            # ---- load weights + rel ---------------------------------------
            wq_sb = consts.tile([P, DM // P, HPC * DK], f16, tag="wq")
            wk_sb = consts.tile([P, DM // P, HPC * DK], f16, tag="wk")
            wv_sb = consts.tile([P, DM // P, HPC * DV], f16, tag="wv")
            for c in range(DM // P):
                nc.sync.dma_start(out=wq_sb[:, c, :], in_=wq[c * P:(c + 1) * P, :])
                nc.sync.dma_start(out=wk_sb[:, c, :], in_=wk[c * P:(c + 1) * P, :])
                nc.sync.dma_start(out=wv_sb[:, c, :], in_=wv[c * P:(c + 1) * P, :])
            wo_sb = consts.tile([P, 2, DM], f16, tag="wo")
            for c in range(2):
                nc.sync.dma_start(out=wo_sb[:, c, :], in_=wo[c * P:(c + 1) * P, :])
            rel_sb = [consts.tile([DK, L], f16, tag=f"rel{h}", name=f"rel_sb{h}")
                      for h in range(HPC)]
            for h in range(HPC):
                nc.sync.dma_start(out=rel_sb[h], in_=rel[h])

            # ---- projections ----------------------------------------------
            # qhT/khT: per head [DK, L] (dk on partitions).  vh: [P, HPC*DV]
            # per key tile (natural).
            qhT = [persist.tile([DK, L], f16, tag=f"qhT{h}", name=f"qhT{h}")
                   for h in range(HPC)]
            khT = [persist.tile([DK, L], f16, tag=f"khT{h}", name=f"khT{h}")
                   for h in range(HPC)]
            vh = [persist.tile([P, HPC * DV], f16, tag=f"vh{t}", name=f"vh{t}")
                  for t in range(TQ)]

            def project_T(dst, w_sb, xT_dram, xtag):
                # dst[h] tiles [DK, L]; contraction over DM in 128-chunks
                for nt in range(L // KC):
                    xT_tiles = []
                    for c in range(DM // P):
                        xt = inload.tile([P, KC], f16, tag=xtag)
                        nc.sync.dma_start(
                            out=xt, in_=xT_dram[c * P:(c + 1) * P,
                                               nt * KC:(nt + 1) * KC])
                        xT_tiles.append(xt)
                    for pair in range(HPC // 2):
                        ps = psS.tile([P, KC], f32, tag="ps_small")
                        for c in range(DM // P):
                            nc.tensor.matmul(
                                ps,
                                w_sb[:, c, pair * 2 * DK:(pair * 2 + 2) * DK],
                                xT_tiles[c],
                                start=(c == 0), stop=(c == DM // P - 1))
                        for i in range(2):
                            h = pair * 2 + i
                            nc.any.tensor_copy(
                                out=dst[h][:, nt * KC:(nt + 1) * KC],
                                in_=ps[i * DK:(i + 1) * DK, :])

            project_T(qhT, wq_sb, qT, "xq")
            project_T(khT, wk_sb, kT, "xk")

            # vh natural: stationary vT tile [dm 128, k 128], moving wv
            for t in range(TQ):
                vT_tiles = []
                for c in range(DM // P):
                    vt = inload.tile([P, P], f16, tag="vTt")
                    nc.sync.dma_start(
                        out=vt, in_=vT[c * P:(c + 1) * P, t * P:(t + 1) * P])
                    vT_tiles.append(vt)
                ps = psS.tile([P, KC], f32, tag="ps_small")
                for c in range(DM // P):
                    nc.tensor.matmul(ps[:, :HPC * DV], vT_tiles[c], wv_sb[:, c, :],
                                     start=(c == 0), stop=(c == DM // P - 1))
                nc.any.tensor_copy(out=vh[t], in_=ps[:, :HPC * DV])

            # ---- per-head R + attention -----------------------------------
            inv_all = [consts.tile([P, TQ], f32, tag=f"inv{h}", name=f"invh{h}")
                       for h in range(HPC)]
            attnTn = [persist.tile([P, L], f16, tag=f"attnTn{i}", name=f"attnTn{i}")
                      for i in range(2)]
            zeros_c = consts.tile([P, KC], f16, tag="zc")
            nc.gpsimd.memset(zeros_c, 0.0)

            def emit_R(h, Rh):
                # zero-fill chunk-0 rows of tiles whose coverage starts later
                zf = [t for t in range(TQ) if _mi_start(t) > 0]
                for t in zf:
                    nc.sync.dma_start(
                        out=Rh[t * P:(t + 1) * P, 0:KC], in_=zeros_c)
                for t in range(TQ):
                    for mi in range(_mi_start(t), L // KC):
                        ps = psS.tile([P, KC], f32, tag="ps_small")
                        nc.tensor.matmul(
                            ps, qhT[h][:, t * P:(t + 1) * P],
                            rel_sb[h][:, mi * KC:(mi + 1) * KC],
                            start=True, stop=True)
                        rt = rstage.tile([P, KC], f16, tag="rt")
                        nc.any.tensor_copy(out=rt, in_=ps)
                        nc.sync.dma_start(
                            out=Rh[t * P:(t + 1) * P, mi * KC:(mi + 1) * KC],
                            in_=rt)

            R_tiles = {}
            R_tiles[0] = rdram.tile([L, L], f16, tag="R", name="R0")
            emit_R(0, R_tiles[0])

            for h in range(HPC):
                Rh = R_tiles[h]
                ptiles = {}
                for t in range(TQ):
                    c = _nchunks(t)
                    gq = t * P
                    segs = [(0, c)] if c <= 3 else [(0, 2), (2, 4)]
                    pt = ptil.tile([P, (L // KC) * KC], f16, tag="pt")
                    sums = small.tile([P, 2], f32, tag="sums")
                    for si, (lo, hi) in enumerate(segs):
                        logits = psL.tile([P, 3 * KC], f32, tag="logits")
                        for ci in range(lo, hi):
                            sl = logits[:, (ci - lo) * KC:(ci - lo + 1) * KC]
                            nc.tensor.matmul(
                                sl, qhT[h][:, gq:gq + P],
                                khT[h][:, ci * KC:(ci + 1) * KC],
                                start=True, stop=False)
                            bt = btiles.tile([P, KC], f16, tag="bt")
                            skew = bass.AP(
                                tensor=Rh.tensor,
                                offset=Rh.offset + gq * L + (L - 1 - gq)
                                + ci * KC,
                                ap=[[L - 1, P], [1, KC]],
                            )
                            nc.sync.dma_start(out=bt, in_=skew)
                            diag = (ci == c - 1)
                            nc.tensor.matmul(sl, ident, bt,
                                             start=False, stop=not diag)
                            if diag:
                                nc.tensor.matmul(sl, ident,
                                                 masks[(gq % KC) // P],
                                                 start=False, stop=True)
                        nc.scalar.activation(
                            out=pt[:, lo * KC:hi * KC],
                            in_=logits[:, :(hi - lo) * KC],
                            func=mybir.ActivationFunctionType.Exp,
                            accum_out=sums[:, si:si + 1])
                    inv = inv_all[h][:, t:t + 1]
                    if len(segs) == 1:
                        nc.vector.reciprocal(out=inv, in_=sums[:, 0:1])
                    else:
                        tot = small.tile([P, 1], f32, tag="tot")
                        nc.vector.tensor_reduce(
                            out=tot, in_=sums, axis=mybir.AxisListType.X,
                            op=mybir.AluOpType.add)
                        nc.vector.reciprocal(out=inv, in_=tot)
                    p_sb = pstage.tile([P, (L // KC) * KC], f32, tag="psb")
                    if (h + t) % 2 == 0:
                        nc.scalar.activation(
                            out=p_sb[:, :c * KC], in_=pt[:, :c * KC],
                            func=mybir.ActivationFunctionType.Copy,
                            scale=inv)
                    else:
                        nc.vector.tensor_scalar_mul(
                            p_sb[:, :c * KC], pt[:, :c * KC], inv)
                    nc.sync.dma_start(
                        out=p_out[h, gq:gq + P, 0:c * KC],
                        in_=p_sb[:, :c * KC])
                    ptiles[t] = pt

                    # ---- PV for the completed 512-row group ----------------
                    if t % 4 == 3:
                        Q = t // 4
                        oT = psS.tile([DV, KC], f32, tag="ps_small")
                        nkc = 4 * (Q + 1)
                        for kc in range(nkc):
                            pTt = pTp.tile([P, KC], f16, tag="pTt")
                            tp = psT.tile([P, KC], f16, tag="ps_t16")
                            for tq in range(4):
                                tt = 4 * Q + tq
                                nc.tensor.transpose(
                                    tp[:, tq * P:(tq + 1) * P],
                                    ptiles[tt][:, kc * P:(kc + 1) * P],
                                    ident)
                            nc.any.tensor_copy(out=pTt, in_=tp)
                            nc.tensor.matmul(
                                oT, vh[kc][:, h * DV:(h + 1) * DV], pTt,
                                start=(kc == 0), stop=(kc == nkc - 1))
                        oT_sb = small.tile([DV, KC], f16, tag="oT_sb")
                        nc.any.tensor_copy(out=oT_sb, in_=oT)
                        for tq in range(4):
                            tt = 4 * Q + tq
                            nat = psT.tile([P, KC], f16, tag="ps_t16")
                            nc.tensor.transpose(
                                nat[:, :DV], oT_sb[:, tq * P:(tq + 1) * P],
                                ident[:DV, :DV])
                            an = small.tile([P, DV], f16, tag="an")
                            nc.vector.tensor_scalar_mul(
                                an, nat[:, :DV], inv_all[h][:, tt:tt + 1])
                            aT = psT.tile([P, KC], f16, tag="ps_t16")
                            nc.tensor.transpose(aT[:DV, :P], an, ident)
                            nc.any.tensor_copy(
                                out=attnTn[h // 2][(h % 2) * DV:(h % 2 + 1) * DV,
                                                  tt * P:(tt + 1) * P],
                                in_=aT[:DV, :P])
                        for tq in range(4):
                            del ptiles[4 * Q + tq]

                if h + 1 < HPC:
                    R_tiles[h + 1] = rdram.tile([L, L], f16, tag="R",
                                                name=f"R{h + 1}")
                    emit_R(h + 1, R_tiles[h + 1])

            # ---- output projection ----------------------------------------
            for t in range(TQ):
                ps = psS.tile([P, KC], f32, tag="ps_small")
                for c in range(2):
                    nc.tensor.matmul(
                        ps, attnTn[c][:, t * P:(t + 1) * P], wo_sb[:, c, :],
                        start=(c == 0), stop=(c == 1))
                o_sb = small.tile([P, DM], f32, tag="o_sb")
                nc.any.tensor_copy(out=o_sb, in_=ps)
                nc.sync.dma_start(out=o_out[t * P:(t + 1) * P, :], in_=o_sb)

    nc.compile()
    return nc


def _get_program():
    if "nc" not in _CACHE:
        _CACHE["nc"] = _build_program()
    return _CACHE["nc"]


def _prep_core_inputs(q, k, v, wq, wk, wv, wo, rel_emb):
    """Build the 8 per-core input dicts (host-side shard + cast)."""
    f16 = np.float16
    in_maps = []
    wq8 = (np.asarray(wq) / math.sqrt(DK)).astype(f16)
    for core in range(NCORES):
        b = core // 2
        hh = core % 2
        hs = slice(hh * HPC * DK, (hh + 1) * HPC * DK)
        in_maps.append({
            "qT": np.ascontiguousarray(np.asarray(q)[b].T).astype(f16),
            "kT": np.ascontiguousarray(np.asarray(k)[b].T).astype(f16),
            "vT": np.ascontiguousarray(np.asarray(v)[b].T).astype(f16),
            "wq": np.ascontiguousarray(wq8[:, hs]),
            "wk": np.ascontiguousarray(np.asarray(wk)[:, hs]).astype(f16),
            "wv": np.ascontiguousarray(np.asarray(wv)[:, hs]).astype(f16),
            "wo": np.ascontiguousarray(np.asarray(wo)[hs, :]).astype(f16),
            "rel": np.ascontiguousarray(
                np.asarray(rel_emb)[hh * HPC:(hh + 1) * HPC]).astype(f16),
        })
    return in_maps


def _assemble(results, q, gamma, beta):
    p = np.empty((B, H, L, L), np.float32)
    out = np.empty((B, L, DM), np.float32)
    qf = np.asarray(q, np.float32)
    for b in range(B):
        r0 = results[b * 2]
        r1 = results[b * 2 + 1]
        p[b, :HPC] = r0["p_out"]
        p[b, HPC:] = r1["p_out"]
        pre = (r0["o_out"].astype(np.float64) + r1["o_out"].astype(np.float64)
               + qf[b].astype(np.float64))
        mu = pre.mean(-1, keepdims=True)
        var = ((pre - mu) ** 2).mean(-1, keepdims=True)
        ln = (pre - mu) / np.sqrt(var + EPS)
        out[b] = (ln * np.asarray(gamma, np.float64)
                  + np.asarray(beta, np.float64)).astype(np.float32)
    return out, p


def kernel(q, k, v, wq, wk, wv, wo, rel_emb, gamma, beta):
    from concourse.bass_utils import run_bass_kernel_spmd

    nc = _get_program()
    in_maps = _prep_core_inputs(q, k, v, wq, wk, wv, wo, rel_emb)
    res = run_bass_kernel_spmd(nc, in_maps, core_ids=list(range(NCORES)))
    return _assemble(res.results, q, gamma, beta)


# revision 13
# speedup vs baseline: 1.0285x; 1.0285x over previous
"""Trainium2 Bass kernel for nn_MultiHeadAttention_49701361549443.

Model (reference.py):
    B, L, DM, H, DK, DV = 4, 2048, 512, 8, 64, 64; MAXLEN = L
    qh/kh/vh = (q|k|v) @ (wq|wk|wv), per-head attention with a causal mask and
    a skewed relative-position bias, softmax -> p, out = p@vh @ wo + residual,
    LayerNorm(eps=1e-6).  Returns (out, p).

Sharding (8 cores): core = batch * 2 + head_half.  Each core handles one
batch and 4 of the 8 heads, all 2048 query rows -- fully uniform SPMD.
The output projection partial sums are reduced on the host, followed by
residual + LayerNorm (trivial numpy work).  p slabs concatenate per head.

Skew trick: the reference's pad+reshape causal skew is equivalent to
    bias[q, k] = R[q, (L-1) - q + k],   R = (q@wq/8) @ rel_emb[h]
which, for R stored row-major [L, L] in DRAM, is a plain 2D strided read with
row stride (L-1).  So R is computed on-chip, round-tripped through DRAM in
fp16, and read back "skewed" with a strided DMA.  Entries with k > q read
garbage from the next row; they are crushed by a -1e4 additive mask before
exp, giving exact 0.0 in the fp16 softmax numerator (matches softmax of
-inf in the reference).
"""

import math

import numpy as np

B, L, DM, H, DK, DV = 4, 2048, 512, 8, 64, 64
HPC = H // 2            # heads per core
NCORES = 8
EPS = 1e-6
P = 128                 # partitions
TQ = L // P             # query tiles of 128 rows
KC = 512                # key-chunk width (one PSUM bank of fp32)
MASKVAL = -10000.0

_CACHE = {}


def _nchunks(t):
    """512-wide key chunks needed by query tile t (causal)."""
    return (t * P) // KC + 1


def _mi_start(t):
    """First 512-wide R column chunk read by tile t (m >= L-128 - t*128)."""
    return max(0, (L - P - t * P)) // KC


def _build_program():
    import concourse.bacc as bacc
    import concourse.bass as bass
    import concourse.tile as tile
    from concourse import mybir
    from concourse.masks import make_identity

    f32 = mybir.dt.float32
    f16 = mybir.dt.float16

    nc = bacc.Bacc(
        "TRN2",
        target_bir_lowering=False,
        debug=False,
        enable_asserts=False,
        num_devices=NCORES,
    )

    # ---- I/O ----------------------------------------------------------------
    qT = nc.dram_tensor("qT", [DM, L], f16, kind="ExternalInput").ap()
    kT = nc.dram_tensor("kT", [DM, L], f16, kind="ExternalInput").ap()
    vT = nc.dram_tensor("vT", [DM, L], f16, kind="ExternalInput").ap()
    wq = nc.dram_tensor("wq", [DM, HPC * DK], f16, kind="ExternalInput").ap()
    wk = nc.dram_tensor("wk", [DM, HPC * DK], f16, kind="ExternalInput").ap()
    wv = nc.dram_tensor("wv", [DM, HPC * DV], f16, kind="ExternalInput").ap()
    wo = nc.dram_tensor("wo", [HPC * DV, DM], f16, kind="ExternalInput").ap()
    rel = nc.dram_tensor("rel", [HPC, DK, L], f16, kind="ExternalInput").ap()

    p_out = nc.dram_tensor("p_out", [HPC, L, L], f32, kind="ExternalOutput").ap()
    o_out = nc.dram_tensor("o_out", [L, DM], f32, kind="ExternalOutput").ap()

    with tile.TileContext(nc) as tc:
        with (
            tc.tile_pool(name="consts", bufs=1) as consts,
            tc.tile_pool(name="persist", bufs=1) as persist,
            tc.tile_pool(name="inload", bufs=4) as inload,
            tc.tile_pool(name="btiles", bufs=3) as btiles,
            tc.tile_pool(name="ptil", bufs=5) as ptil,
            tc.tile_pool(name="pT", bufs=3) as pTp,
            tc.tile_pool(name="small", bufs=4) as small,
            tc.tile_pool(name="rstage", bufs=2) as rstage,
            tc.tile_pool(name="pstage", bufs=2) as pstage,
            tc.tile_pool(name="psL", bufs=2, space="PSUM") as psL,
            tc.tile_pool(name="psS", bufs=2, space="PSUM") as psS,
            tc.tile_pool(name="psT", bufs=2, space="PSUM") as psT,
            tc.tile_pool(name="rdram", bufs=2, space="DRAM") as rdram,
        ):
            # ---- constants ------------------------------------------------
            ident = consts.tile([P, P], f16)
            make_identity(nc, ident)
            masks = []
            for d in (0, 128, 256, 384):
                m = consts.tile([P, KC], f16, tag=f"mask{d}", name=f"mask{d}")
                nc.gpsimd.memset(m, 0.0)
                # keep 0.0 where (d + p - j) >= 0  i.e. j <= d+p, else MASKVAL
                nc.gpsimd.affine_select(
                    out=m, in_=m,
                    compare_op=mybir.AluOpType.is_ge,
                    fill=MASKVAL, base=d,
                    pattern=[[-1, KC]], channel_multiplier=1,
                )
                masks.append(m)

            evac_ctr = [0]

            def evac(out, in_):
                evac_ctr[0] += 1
                if evac_ctr[0] % 2 == 0:
                    nc.vector.tensor_copy(out=out, in_=in_)
                else:
                    nc.scalar.activation(
                        out=out, in_=in_,
                        func=mybir.ActivationFunctionType.Copy)

            # ---- load weights + rel ---------------------------------------
            wq_sb = consts.tile([P, DM // P, HPC * DK], f16, tag="wq")
            wk_sb = consts.tile([P, DM // P, HPC * DK], f16, tag="wk")
            wv_sb = consts.tile([P, DM // P, HPC * DV], f16, tag="wv")
            for c in range(DM // P):
                nc.sync.dma_start(out=wq_sb[:, c, :], in_=wq[c * P:(c + 1) * P, :])
                nc.sync.dma_start(out=wk_sb[:, c, :], in_=wk[c * P:(c + 1) * P, :])
                nc.sync.dma_start(out=wv_sb[:, c, :], in_=wv[c * P:(c + 1) * P, :])
            wo_sb = consts.tile([P, 2, DM], f16, tag="wo")
            for c in range(2):
                nc.sync.dma_start(out=wo_sb[:, c, :], in_=wo[c * P:(c + 1) * P, :])
            rel_sb = [consts.tile([DK, L], f16, tag=f"rel{h}", name=f"rel_sb{h}")
                      for h in range(HPC)]
            for h in range(HPC):
                nc.sync.dma_start(out=rel_sb[h], in_=rel[h])

            # ---- projections ----------------------------------------------
            # qhT/khT: per head [DK, L] (dk on partitions).  vh: [P, HPC*DV]
            # per key tile (natural).
            qhT = [persist.tile([DK, L], f16, tag=f"qhT{h}", name=f"qhT{h}")
                   for h in range(HPC)]
            khT = [persist.tile([DK, L], f16, tag=f"khT{h}", name=f"khT{h}")
                   for h in range(HPC)]
            vh = [persist.tile([P, HPC * DV], f16, tag=f"vh{t}", name=f"vh{t}")
                  for t in range(TQ)]

            HL = L // 2

            def project_T(dst, w_sb, xT_dram, xtag):
                # dst[h] tiles [DK, L]; contraction over DM in 128-chunks
                for half in range(2):
                    xT_tiles = []
                    for c in range(DM // P):
                        xt = inload.tile([P, HL], f16, tag="xin",
                                         name=f"{xtag}{half}_{c}")
                        nc.sync.dma_start(
                            out=xt,
                            in_=xT_dram[c * P:(c + 1) * P,
                                        half * HL:(half + 1) * HL])
                        xT_tiles.append(xt)
                    for nt in range(HL // KC):
                        for pair in range(HPC // 2):
                            ps = psS.tile([P, KC], f32, tag="ps_small")
                            for c in range(DM // P):
                                nc.tensor.matmul(
                                    ps,
                                    w_sb[:, c,
                                         pair * 2 * DK:(pair * 2 + 2) * DK],
                                    xT_tiles[c][:, nt * KC:(nt + 1) * KC],
                                    start=(c == 0), stop=(c == DM // P - 1))
                            for i in range(2):
                                h = pair * 2 + i
                                evac(dst[h][:, half * HL + nt * KC:
                                            half * HL + (nt + 1) * KC],
                                     ps[i * DK:(i + 1) * DK, :])

            project_T(qhT, wq_sb, qT, "xq")
            project_T(khT, wk_sb, kT, "xk")

            # vh natural: stationary vT tile [dm 128, k 128], moving wv
            for half in range(2):
                vT_sb = []
                for c in range(DM // P):
                    vt = inload.tile([P, HL], f16, tag="xin",
                                     name=f"vT{half}_{c}")
                    nc.sync.dma_start(
                        out=vt, in_=vT[c * P:(c + 1) * P,
                                       half * HL:(half + 1) * HL])
                    vT_sb.append(vt)
                for t in range(half * TQ // 2, (half + 1) * TQ // 2):
                    ps = psS.tile([P, KC], f32, tag="ps_small")
                    for c in range(DM // P):
                        nc.tensor.matmul(
                            ps[:, :HPC * DV],
                            vT_sb[c][:, t * P - half * HL:
                                     (t + 1) * P - half * HL],
                            wv_sb[:, c, :],
                            start=(c == 0), stop=(c == DM // P - 1))
                    evac(vh[t], ps[:, :HPC * DV])

            # ---- per-head R + attention -----------------------------------
            inv_all = [consts.tile([P, TQ], f32, tag=f"inv{h}", name=f"invh{h}")
                       for h in range(HPC)]
            attnTn = [persist.tile([P, L], f16, tag=f"attnTn{i}", name=f"attnTn{i}")
                      for i in range(2)]
            zeros_c = consts.tile([P, KC], f16, tag="zc")
            nc.gpsimd.memset(zeros_c, 0.0)

            def emit_R(h, Rh):
                # zero-fill chunk-0 rows of tiles whose coverage starts later
                zf = [t for t in range(TQ) if _mi_start(t) > 0]
                for t in zf:
                    nc.gpsimd.dma_start(
                        out=Rh[t * P:(t + 1) * P, 0:KC], in_=zeros_c)
                for t in range(TQ):
                    mi0 = _mi_start(t)
                    nm = L // KC - mi0
                    rt = rstage.tile([P, (L // KC) * KC], f16, tag="rt")
                    for j in range(nm):
                        ps = psS.tile([P, KC], f32, tag="ps_small")
                        nc.tensor.matmul(
                            ps, qhT[h][:, t * P:(t + 1) * P],
                            rel_sb[h][:, (mi0 + j) * KC:(mi0 + j + 1) * KC],
                            start=True, stop=True)
                        evac(rt[:, j * KC:(j + 1) * KC], ps)
                    nc.gpsimd.dma_start(
                        out=Rh[t * P:(t + 1) * P, mi0 * KC:],
                        in_=rt[:, :nm * KC])

            R_tiles = {}
            R_tiles[0] = rdram.tile([L, L], f16, tag="R", name="R0")
            emit_R(0, R_tiles[0])

            for h in range(HPC):
                Rh = R_tiles[h]
                ptiles = {}
                for t in range(TQ):
                    c = _nchunks(t)
                    gq = t * P
                    segs = [(i, min(i + 2, c)) for i in range(0, c, 2)]
                    pt = ptil.tile([P, (L // KC) * KC], f16, tag="pt")
                    sums = small.tile([P, 2], f32, tag="sums")
                    bt = btiles.tile([P, (L // KC) * KC], f16, tag="bt")
                    skew = bass.AP(
                        tensor=Rh.tensor,
                        offset=Rh.offset + gq * L + (L - 1 - gq),
                        ap=[[L - 1, P], [1, c * KC]],
                    )
                    nc.gpsimd.dma_start(out=bt[:, :c * KC], in_=skew)
                    for si, (lo, hi) in enumerate(segs):
                        logits = psL.tile([P, 2 * KC], f32, tag="logits")
                        for ci in range(lo, hi):
                            sl = logits[:, (ci - lo) * KC:(ci - lo + 1) * KC]
                            nc.tensor.matmul(
                                sl, qhT[h][:, gq:gq + P],
                                khT[h][:, ci * KC:(ci + 1) * KC],
                                start=True, stop=False)
                            diag = (ci == c - 1)
                            nc.tensor.matmul(
                                sl, ident, bt[:, ci * KC:(ci + 1) * KC],
                                start=False, stop=not diag)
                            if diag:
                                nc.tensor.matmul(sl, ident,
                                                 masks[(gq % KC) // P],
                                                 start=False, stop=True)
                        nc.scalar.activation(
                            out=pt[:, lo * KC:hi * KC],
                            in_=logits[:, :(hi - lo) * KC],
                            func=mybir.ActivationFunctionType.Exp,
                            accum_out=sums[:, si:si + 1])
                    inv = inv_all[h][:, t:t + 1]
                    if len(segs) == 1:
                        nc.vector.reciprocal(out=inv, in_=sums[:, 0:1])
                    else:
                        tot = small.tile([P, 1], f32, tag="tot")
                        nc.vector.tensor_reduce(
                            out=tot, in_=sums, axis=mybir.AxisListType.X,
                            op=mybir.AluOpType.add)
                        nc.vector.reciprocal(out=inv, in_=tot)
                    p_sb = pstage.tile([P, (L // KC) * KC], f32, tag="psb")
                    if (h + t) % 2 == 0:
                        nc.scalar.activation(
                            out=p_sb[:, :c * KC], in_=pt[:, :c * KC],
                            func=mybir.ActivationFunctionType.Copy,
                            scale=inv)
                    else:
                        nc.vector.tensor_scalar_mul(
                            p_sb[:, :c * KC], pt[:, :c * KC], inv)
                    nc.sync.dma_start(
                        out=p_out[h, gq:gq + P, 0:c * KC],
                        in_=p_sb[:, :c * KC])
                    ptiles[t] = pt

                    # ---- PV for the completed 512-row group ----------------
                    if t % 4 == 3:
                        Q = t // 4
                        oT = psS.tile([DV, KC], f32, tag="ps_small")
                        nkc = 4 * (Q + 1)
                        for kc in range(nkc):
                            pTt = pTp.tile([P, KC], f16, tag="pTt")
                            tp = psT.tile([P, KC], f16, tag="ps_t16")
                            for tq in range(4):
                                tt = 4 * Q + tq
                                nc.tensor.transpose(
                                    tp[:, tq * P:(tq + 1) * P],
                                    ptiles[tt][:, kc * P:(kc + 1) * P],
                                    ident)
                            nc.vector.tensor_copy(
                                out=pTt[:, :KC // 2], in_=tp[:, :KC // 2])
                            nc.scalar.activation(
                                out=pTt[:, KC // 2:], in_=tp[:, KC // 2:],
                                func=mybir.ActivationFunctionType.Copy)
                            nc.tensor.matmul(
                                oT, vh[kc][:, h * DV:(h + 1) * DV], pTt,
                                start=(kc == 0), stop=(kc == nkc - 1))
                        oT_sb = small.tile([DV, KC], f16, tag="oT_sb")
                        evac(oT_sb, oT)
                        for tq in range(4):
                            tt = 4 * Q + tq
                            nat = psT.tile([P, KC], f16, tag="ps_t16")
                            nc.tensor.transpose(
                                nat[:, :DV], oT_sb[:, tq * P:(tq + 1) * P],
                                ident[:DV, :DV])
                            an = small.tile([P, DV], f16, tag="an")
                            nc.vector.tensor_scalar_mul(
                                an, nat[:, :DV], inv_all[h][:, tt:tt + 1])
                            aT = psT.tile([P, KC], f16, tag="ps_t16")
                            nc.tensor.transpose(aT[:DV, :P], an, ident)
                            evac(attnTn[h // 2][(h % 2) * DV:(h % 2 + 1) * DV,
                                                tt * P:(tt + 1) * P],
                                 aT[:DV, :P])
                        for tq in range(4):
                            del ptiles[4 * Q + tq]

                if h + 1 < HPC:
                    R_tiles[h + 1] = rdram.tile([L, L], f16, tag="R",
                                                name=f"R{h + 1}")
                    emit_R(h + 1, R_tiles[h + 1])

            # ---- output projection ----------------------------------------
            for t in range(TQ):
                ps = psS.tile([P, KC], f32, tag="ps_small")
                for c in range(2):
                    nc.tensor.matmul(
                        ps, attnTn[c][:, t * P:(t + 1) * P], wo_sb[:, c, :],
                        start=(c == 0), stop=(c == 1))
                o_sb = small.tile([P, DM], f32, tag="o_sb")
                evac(o_sb, ps)
                nc.sync.dma_start(out=o_out[t * P:(t + 1) * P, :], in_=o_sb)

    nc.compile()
    return nc


def _get_program():
    if "nc" not in _CACHE:
        _CACHE["nc"] = _build_program()
    return _CACHE["nc"]


def _prep_core_inputs(q, k, v, wq, wk, wv, wo, rel_emb):
    """Build the 8 per-core input dicts (host-side shard + cast)."""
    f16 = np.float16
    in_maps = []
    wq8 = (np.asarray(wq) / math.sqrt(DK)).astype(f16)
    for core in range(NCORES):
        b = core // 2
        hh = core % 2
        hs = slice(hh * HPC * DK, (hh + 1) * HPC * DK)
        in_maps.append({
            "qT": np.ascontiguousarray(np.asarray(q)[b].T).astype(f16),
            "kT": np.ascontiguousarray(np.asarray(k)[b].T).astype(f16),
            "vT": np.ascontiguousarray(np.asarray(v)[b].T).astype(f16),
            "wq": np.ascontiguousarray(wq8[:, hs]),
            "wk": np.ascontiguousarray(np.asarray(wk)[:, hs]).astype(f16),
            "wv": np.ascontiguousarray(np.asarray(wv)[:, hs]).astype(f16),
            "wo": np.ascontiguousarray(np.asarray(wo)[hs, :]).astype(f16),
            "rel": np.ascontiguousarray(
                np.asarray(rel_emb)[hh * HPC:(hh + 1) * HPC]).astype(f16),
        })
    return in_maps


def _assemble(results, q, gamma, beta):
    p = np.empty((B, H, L, L), np.float32)
    out = np.empty((B, L, DM), np.float32)
    qf = np.asarray(q, np.float32)
    for b in range(B):
        r0 = results[b * 2]
        r1 = results[b * 2 + 1]
        p[b, :HPC] = r0["p_out"]
        p[b, HPC:] = r1["p_out"]
        pre = (r0["o_out"].astype(np.float64) + r1["o_out"].astype(np.float64)
               + qf[b].astype(np.float64))
        mu = pre.mean(-1, keepdims=True)
        var = ((pre - mu) ** 2).mean(-1, keepdims=True)
        ln = (pre - mu) / np.sqrt(var + EPS)
        out[b] = (ln * np.asarray(gamma, np.float64)
                  + np.asarray(beta, np.float64)).astype(np.float32)
    return out, p


def kernel(q, k, v, wq, wk, wv, wo, rel_emb, gamma, beta):
    from concourse.bass_utils import run_bass_kernel_spmd

    nc = _get_program()
    in_maps = _prep_core_inputs(q, k, v, wq, wk, wv, wo, rel_emb)
    res = run_bass_kernel_spmd(nc, in_maps, core_ids=list(range(NCORES)))
    return _assemble(res.results, q, gamma, beta)


# revision 17
# speedup vs baseline: 1.1135x; 1.0826x over previous
"""Trainium2 Bass kernel for nn_MultiHeadAttention_49701361549443.

Model (reference.py):
    B, L, DM, H, DK, DV = 4, 2048, 512, 8, 64, 64; MAXLEN = L
    qh/kh/vh = (q|k|v) @ (wq|wk|wv), per-head attention with a causal mask and
    a skewed relative-position bias, softmax -> p, out = p@vh @ wo + residual,
    LayerNorm(eps=1e-6).  Returns (out, p).

Sharding (8 cores): core = batch * 2 + head_half.  Each core handles one
batch and 4 of the 8 heads, all 2048 query rows -- fully uniform SPMD.
The output projection partial sums are reduced on the host, followed by
residual + LayerNorm (trivial numpy work).  p slabs concatenate per head.

Skew trick: the reference's pad+reshape causal skew is equivalent to
    bias[q, k] = R[q, (L-1) - q + k],   R = (q@wq/8) @ rel_emb[h]
which, for R stored row-major [L, L] in DRAM, is a plain 2D strided read with
row stride (L-1).  So R is computed on-chip, round-tripped through DRAM in
fp16, and read back "skewed" with a strided DMA.  Entries with k > q read
garbage from the next row; they are crushed by a -1e4 additive mask before
exp, giving exact 0.0 in the fp16 softmax numerator (matches softmax of
-inf in the reference).
"""

import math

import numpy as np

B, L, DM, H, DK, DV = 4, 2048, 512, 8, 64, 64
HPC = H // 2            # heads per core
NCORES = 8
EPS = 1e-6
P = 128                 # partitions
TQ = L // P             # query tiles of 128 rows
KC = 512                # key-chunk width (one PSUM bank of fp32)
MASKVAL = -10000.0
RW = 2560               # R_dram row pitch: L + 512 pad of MASKVAL

_CACHE = {}


def _nchunks(t):
    """512-wide key chunks needed by query tile t (causal)."""
    return (t * P) // KC + 1


def _mi_start(t):
    """First 512-wide R column chunk read by tile t (m >= L-128 - t*128)."""
    return max(0, (L - P - t * P)) // KC


def _build_program():
    import concourse.bacc as bacc
    import concourse.bass as bass
    import concourse.tile as tile
    from concourse import mybir
    from concourse.masks import make_identity

    f32 = mybir.dt.float32
    f16 = mybir.dt.float16

    nc = bacc.Bacc(
        "TRN2",
        target_bir_lowering=False,
        debug=False,
        enable_asserts=False,
        num_devices=NCORES,
    )

    # ---- I/O ----------------------------------------------------------------
    qT = nc.dram_tensor("qT", [DM, L], f16, kind="ExternalInput").ap()
    kT = nc.dram_tensor("kT", [DM, L], f16, kind="ExternalInput").ap()
    vT = nc.dram_tensor("vT", [DM, L], f16, kind="ExternalInput").ap()
    wq = nc.dram_tensor("wq", [DM, HPC * DK], f16, kind="ExternalInput").ap()
    wk = nc.dram_tensor("wk", [DM, HPC * DK], f16, kind="ExternalInput").ap()
    wv = nc.dram_tensor("wv", [DM, HPC * DV], f16, kind="ExternalInput").ap()
    wo = nc.dram_tensor("wo", [HPC * DV, DM], f16, kind="ExternalInput").ap()
    rel = nc.dram_tensor("rel", [HPC, DK, L], f16, kind="ExternalInput").ap()

    p_out = nc.dram_tensor("p_out", [HPC, L, L], f32, kind="ExternalOutput").ap()
    o_out = nc.dram_tensor("o_out", [L, DM], f32, kind="ExternalOutput").ap()

    with tile.TileContext(nc) as tc:
        with (
            tc.tile_pool(name="consts", bufs=1) as consts,
            tc.tile_pool(name="persist", bufs=1) as persist,
            tc.tile_pool(name="inload", bufs=4) as inload,
            tc.tile_pool(name="btiles", bufs=3) as btiles,
            tc.tile_pool(name="ptil", bufs=5) as ptil,
            tc.tile_pool(name="pT", bufs=3) as pTp,
            tc.tile_pool(name="small", bufs=4) as small,
            tc.tile_pool(name="rstage", bufs=2) as rstage,
            tc.tile_pool(name="pstage", bufs=2) as pstage,
            tc.tile_pool(name="psL", bufs=2, space="PSUM") as psL,
            tc.tile_pool(name="psS", bufs=2, space="PSUM") as psS,
            tc.tile_pool(name="psT", bufs=2, space="PSUM") as psT,
            tc.tile_pool(name="rdram", bufs=2, space="DRAM") as rdram,
        ):
            # ---- constants ------------------------------------------------
            ident = consts.tile([P, P], f16)
            make_identity(nc, ident)
            negm = consts.tile([P, KC], f16, tag="negm", name="negm")
            nc.gpsimd.memset(negm, MASKVAL)

            evac_ctr = [0]

            def evac(out, in_):
                evac_ctr[0] += 1
                if evac_ctr[0] % 2 == 0:
                    nc.vector.tensor_copy(out=out, in_=in_)
                else:
                    nc.scalar.activation(
                        out=out, in_=in_,
                        func=mybir.ActivationFunctionType.Copy)

            # ---- load weights + rel ---------------------------------------
            wq_sb = consts.tile([P, DM // P, HPC * DK], f16, tag="wq")
            wk_sb = consts.tile([P, DM // P, HPC * DK], f16, tag="wk")
            wv_sb = consts.tile([P, DM // P, HPC * DV], f16, tag="wv")
            for c in range(DM // P):
                nc.sync.dma_start(out=wq_sb[:, c, :], in_=wq[c * P:(c + 1) * P, :])
                nc.sync.dma_start(out=wk_sb[:, c, :], in_=wk[c * P:(c + 1) * P, :])
                nc.sync.dma_start(out=wv_sb[:, c, :], in_=wv[c * P:(c + 1) * P, :])
            wo_sb = consts.tile([P, 2, DM], f16, tag="wo")
            for c in range(2):
                nc.sync.dma_start(out=wo_sb[:, c, :], in_=wo[c * P:(c + 1) * P, :])
            rel_sb = [consts.tile([P, L], f16, tag=f"rel{h}", name=f"rel_sb{h}")
                      for h in range(HPC)]
            for h in range(HPC):
                nc.sync.dma_start(out=rel_sb[h][:DK, :], in_=rel[h])
                nc.sync.dma_start(out=rel_sb[h][DK:, :], in_=rel[h])

            # ---- projections ----------------------------------------------
            # qhT/khT: per head [DK, L] (dk on partitions).  vh: [P, HPC*DV]
            # per key tile (natural).
            # [128, L]: the head's 64-row block duplicated at partitions
            # 0-63 and 64-127, so two key chunks stream concurrently via
            # tile_position row groups.
            qhT = [persist.tile([P, L], f16, tag=f"qhT{h}", name=f"qhT{h}")
                   for h in range(HPC)]
            khT = [persist.tile([P, L], f16, tag=f"khT{h}", name=f"khT{h}")
                   for h in range(HPC)]
            vh = [persist.tile([P, HPC * DV], f16, tag=f"vh{t}", name=f"vh{t}")
                  for t in range(TQ)]

            HL = L // 2

            def project_T(dst, w_sb, xT_dram, xtag):
                # dst[h] tiles [DK, L]; contraction over DM in 128-chunks
                for half in range(2):
                    xT_tiles = []
                    for c in range(DM // P):
                        xt = inload.tile([P, HL], f16, tag="xin",
                                         name=f"{xtag}{half}_{c}")
                        nc.sync.dma_start(
                            out=xt,
                            in_=xT_dram[c * P:(c + 1) * P,
                                        half * HL:(half + 1) * HL])
                        xT_tiles.append(xt)
                    for nt in range(HL // KC):
                        for pair in range(HPC // 2):
                            ps = psS.tile([P, KC], f32, tag="ps_small")
                            for c in range(DM // P):
                                nc.tensor.matmul(
                                    ps,
                                    w_sb[:, c,
                                         pair * 2 * DK:(pair * 2 + 2) * DK],
                                    xT_tiles[c][:, nt * KC:(nt + 1) * KC],
                                    start=(c == 0), stop=(c == DM // P - 1))
                            for i in range(2):
                                h = pair * 2 + i
                                sl = slice(half * HL + nt * KC,
                                           half * HL + (nt + 1) * KC)
                                evac(dst[h][:DK, sl], ps[i * DK:(i + 1) * DK, :])
                                evac(dst[h][DK:, sl], ps[i * DK:(i + 1) * DK, :])

            project_T(qhT, wq_sb, qT, "xq")
            project_T(khT, wk_sb, kT, "xk")

            # vh natural: stationary vT tile [dm 128, k 128], moving wv
            for half in range(2):
                vT_sb = []
                for c in range(DM // P):
                    vt = inload.tile([P, HL], f16, tag="xin",
                                     name=f"vT{half}_{c}")
                    nc.sync.dma_start(
                        out=vt, in_=vT[c * P:(c + 1) * P,
                                       half * HL:(half + 1) * HL])
                    vT_sb.append(vt)
                for t in range(half * TQ // 2, (half + 1) * TQ // 2):
                    ps = psS.tile([P, KC], f32, tag="ps_small")
                    for c in range(DM // P):
                        nc.tensor.matmul(
                            ps[:, :HPC * DV],
                            vT_sb[c][:, t * P - half * HL:
                                     (t + 1) * P - half * HL],
                            wv_sb[:, c, :],
                            start=(c == 0), stop=(c == DM // P - 1))
                    evac(vh[t], ps[:, :HPC * DV])

            # ---- per-head R + attention -----------------------------------
            inv_all = [consts.tile([P, TQ], f32, tag=f"inv{h}", name=f"invh{h}")
                       for h in range(HPC)]
            attnTn = [persist.tile([P, L], f16, tag=f"attnTn{i}", name=f"attnTn{i}")
                      for i in range(2)]

            def emit_R(h, Rh):
                # fill the 512-wide pad columns with MASKVAL (one DMA; the
                # source tile is re-read 16x -- constant data, order moot)
                for blk in range(L // P):
                    nc.gpsimd.dma_start(
                        out=Rh[blk * P:(blk + 1) * P, L:RW], in_=negm)
                for t in range(TQ):
                    mi0 = _mi_start(t)
                    nm = L // KC - mi0
                    rt = rstage.tile([P, (L // KC) * KC], f16, tag="rt")
                    for j in range(nm):
                        base = (j % 2) * DK
                        ps = psS.tile([P, KC], f32, tag="ps_small")
                        nc.tensor.matmul(
                            ps,
                            qhT[h][base:base + DK, t * P:(t + 1) * P],
                            rel_sb[h][base:base + DK,
                                      (mi0 + j) * KC:(mi0 + j + 1) * KC],
                            start=True, stop=True)
                        evac(rt[:, j * KC:(j + 1) * KC], ps)
                    nc.gpsimd.dma_start(
                        out=Rh[t * P:(t + 1) * P, mi0 * KC:L],
                        in_=rt[:, :nm * KC])

            R_tiles = {}
            R_tiles[0] = rdram.tile([L, RW], f16, tag="R", name="R0")
            emit_R(0, R_tiles[0])

            for h in range(HPC):
                Rh = R_tiles[h]
                ptiles = {}
                for t in range(TQ):
                    c = _nchunks(t)
                    gq = t * P
                    segs = [(i, min(i + 2, c)) for i in range(0, c, 2)]
                    pt = ptil.tile([P, (L // KC) * KC], f16, tag="pt")
                    sums = small.tile([P, 2], f32, tag="sums")
                    bt = btiles.tile([P, (L // KC) * KC], f16, tag="bt")
                    # skewed read incl. the MASKVAL pad for k > q (diag chunk)
                    skew = bass.AP(
                        tensor=Rh.tensor,
                        offset=Rh.offset + gq * RW + (L - 1 - gq),
                        ap=[[RW - 1, P], [1, c * KC]],
                    )
                    nc.gpsimd.dma_start(out=bt[:, :c * KC], in_=skew)
                    for si, (lo, hi) in enumerate(segs):
                        logits = psL.tile([P, 2 * KC], f32, tag="logits")
                        for ci in range(lo, hi):
                            base = ((ci - lo) % 2) * DK
                            nc.tensor.matmul(
                                logits[:, (ci - lo) * KC:(ci - lo + 1) * KC],
                                qhT[h][base:base + DK, gq:gq + P],
                                khT[h][base:base + DK,
                                       ci * KC:(ci + 1) * KC],
                                start=True, stop=False)
                        for ci in range(lo, hi):
                            nc.tensor.matmul(
                                logits[:, (ci - lo) * KC:(ci - lo + 1) * KC],
                                ident, bt[:, ci * KC:(ci + 1) * KC],
                                start=False, stop=True)
                        nc.scalar.activation(
                            out=pt[:, lo * KC:hi * KC],
                            in_=logits[:, :(hi - lo) * KC],
                            func=mybir.ActivationFunctionType.Exp,
                            accum_out=sums[:, si:si + 1])
                    inv = inv_all[h][:, t:t + 1]
                    if len(segs) == 1:
                        nc.vector.reciprocal(out=inv, in_=sums[:, 0:1])
                    else:
                        tot = small.tile([P, 1], f32, tag="tot")
                        nc.vector.tensor_reduce(
                            out=tot, in_=sums, axis=mybir.AxisListType.X,
                            op=mybir.AluOpType.add)
                        nc.vector.reciprocal(out=inv, in_=tot)
                    p_sb = pstage.tile([P, (L // KC) * KC], f32, tag="psb")
                    nc.vector.tensor_scalar_mul(
                        p_sb[:, :c * KC], pt[:, :c * KC], inv)
                    nc.sync.dma_start(
                        out=p_out[h, gq:gq + P, 0:c * KC],
                        in_=p_sb[:, :c * KC])
                    ptiles[t] = pt

                    # ---- PV for the completed 512-row group ----------------
                    if t % 4 == 3:
                        Q = t // 4
                        oT = psS.tile([DV, KC], f32, tag="ps_small")
                        nkc = 4 * (Q + 1)
                        for kc in range(nkc):
                            pTt = pTp.tile([P, KC], f16, tag="pTt")
                            tp = psT.tile([P, KC], f16, tag="ps_t16")
                            # block (tt, kc) is identically zero when kc > tt
                            # (above the causal diagonal): memset instead of
                            # transpose+copy.
                            nz0 = max(0, kc - 4 * Q)   # first nonzero tq
                            if nz0 > 0:
                                nc.gpsimd.memset(pTt[:, :nz0 * P], 0.0)
                            for tq in range(nz0, 4):
                                tt = 4 * Q + tq
                                nc.tensor.transpose(
                                    tp[:, tq * P:(tq + 1) * P],
                                    ptiles[tt][:, kc * P:(kc + 1) * P],
                                    ident)
                            w = (4 - nz0) * P
                            h1 = nz0 * P + w // 2
                            nc.vector.tensor_copy(
                                out=pTt[:, nz0 * P:h1], in_=tp[:, nz0 * P:h1])
                            nc.scalar.activation(
                                out=pTt[:, h1:], in_=tp[:, h1:],
                                func=mybir.ActivationFunctionType.Copy)
                            nc.tensor.matmul(
                                oT, vh[kc][:, h * DV:(h + 1) * DV], pTt,
                                start=(kc == 0), stop=(kc == nkc - 1))
                        oT_sb = small.tile([DV, KC], f16, tag="oT_sb")
                        evac(oT_sb, oT)
                        for tq in range(4):
                            tt = 4 * Q + tq
                            nat = psT.tile([P, KC], f16, tag="ps_t16")
                            nc.tensor.transpose(
                                nat[:, :DV], oT_sb[:, tq * P:(tq + 1) * P],
                                ident[:DV, :DV])
                            an = small.tile([P, DV], f16, tag="an")
                            nc.vector.tensor_scalar_mul(
                                an, nat[:, :DV], inv_all[h][:, tt:tt + 1])
                            aT = psT.tile([P, KC], f16, tag="ps_t16")
                            nc.tensor.transpose(aT[:DV, :P], an, ident)
                            evac(attnTn[h // 2][(h % 2) * DV:(h % 2 + 1) * DV,
                                                tt * P:(tt + 1) * P],
                                 aT[:DV, :P])
                        for tq in range(4):
                            del ptiles[4 * Q + tq]

                if h + 1 < HPC:
                    R_tiles[h + 1] = rdram.tile([L, RW], f16, tag="R",
                                                name=f"R{h + 1}")
                    emit_R(h + 1, R_tiles[h + 1])

            # ---- output projection ----------------------------------------
            for t in range(TQ):
                ps = psS.tile([P, KC], f32, tag="ps_small")
                for c in range(2):
                    nc.tensor.matmul(
                        ps, attnTn[c][:, t * P:(t + 1) * P], wo_sb[:, c, :],
                        start=(c == 0), stop=(c == 1))
                o_sb = small.tile([P, DM], f32, tag="o_sb")
                evac(o_sb, ps)
                nc.sync.dma_start(out=o_out[t * P:(t + 1) * P, :], in_=o_sb)

    nc.compile()
    return nc


def _get_program():
    if "nc" not in _CACHE:
        _CACHE["nc"] = _build_program()
    return _CACHE["nc"]


def _prep_core_inputs(q, k, v, wq, wk, wv, wo, rel_emb):
    """Build the 8 per-core input dicts (host-side shard + cast)."""
    f16 = np.float16
    in_maps = []
    wq8 = (np.asarray(wq) / math.sqrt(DK)).astype(f16)
    for core in range(NCORES):
        b = core // 2
        hh = core % 2
        hs = slice(hh * HPC * DK, (hh + 1) * HPC * DK)
        in_maps.append({
            "qT": np.ascontiguousarray(np.asarray(q)[b].T).astype(f16),
            "kT": np.ascontiguousarray(np.asarray(k)[b].T).astype(f16),
            "vT": np.ascontiguousarray(np.asarray(v)[b].T).astype(f16),
            "wq": np.ascontiguousarray(wq8[:, hs]),
            "wk": np.ascontiguousarray(np.asarray(wk)[:, hs]).astype(f16),
            "wv": np.ascontiguousarray(np.asarray(wv)[:, hs]).astype(f16),
            "wo": np.ascontiguousarray(np.asarray(wo)[hs, :]).astype(f16),
            "rel": np.ascontiguousarray(
                np.asarray(rel_emb)[hh * HPC:(hh + 1) * HPC]).astype(f16),
        })
    return in_maps


def _assemble(results, q, gamma, beta):
    p = np.empty((B, H, L, L), np.float32)
    out = np.empty((B, L, DM), np.float32)
    qf = np.asarray(q, np.float32)
    for b in range(B):
        r0 = results[b * 2]
        r1 = results[b * 2 + 1]
        p[b, :HPC] = r0["p_out"]
        p[b, HPC:] = r1["p_out"]
        pre = (r0["o_out"].astype(np.float64) + r1["o_out"].astype(np.float64)
               + qf[b].astype(np.float64))
        mu = pre.mean(-1, keepdims=True)
        var = ((pre - mu) ** 2).mean(-1, keepdims=True)
        ln = (pre - mu) / np.sqrt(var + EPS)
        out[b] = (ln * np.asarray(gamma, np.float64)
                  + np.asarray(beta, np.float64)).astype(np.float32)
    return out, p


def kernel(q, k, v, wq, wk, wv, wo, rel_emb, gamma, beta):
    from concourse.bass_utils import run_bass_kernel_spmd

    nc = _get_program()
    in_maps = _prep_core_inputs(q, k, v, wq, wk, wv, wo, rel_emb)
    res = run_bass_kernel_spmd(nc, in_maps, core_ids=list(range(NCORES)))
    return _assemble(res.results, q, gamma, beta)
